# revision 6
# baseline (speedup 1.0000x reference)
"""BiLSTM encoder (B=64, T=256, D=H=1024, L=2) on 8 Trainium2 NeuronCores.

Sharding: cores 0-3 run the forward direction, cores 4-7 the backward
direction (backward cores get time-reversed inputs so the program is
identical). Within each 4-core group, the 4H=4096 gate columns are sharded
into chunks of 1024 (= 256 h-indices x 4 gates); every core holds the full
batch of 64. Each recurrence step all-gathers the per-core 256-row h chunk
so every core has the full h for the next step's GEMM. Layer 1 runs
wavefronted one step behind layer 0 and consumes the layer-0 all-gather
output directly as its input GEMM operand.

Layout notes:
- All GEMMs are weight-stationary: out^T[gate_cols, batch] = W_block^T @ rhs
  with W blocks [128k, 128m] as lhsT and rhs [128, 64] slices streaming.
- Per-step PSUM tile [128, 512] holds 8 m-tiles in column order
  [i0 i1 f0 f1 o0 o1 g0 g1] (i/f/o/g gates, 2 h-blocks of 128 each).
- Cell state c is fp32, h and activations bf16.
"""

import sys

sys.path.insert(0, "/opt/trn_rl_repo")

import numpy as np
import ml_dtypes

import concourse.bass as bass
import concourse.bacc as bacc
import concourse.mybir as mybir
import concourse.tile as tile
from concourse.bass_utils import run_bass_kernel_spmd

F32 = mybir.dt.float32
BF16 = mybir.dt.bfloat16
AF = mybir.ActivationFunctionType
ALU = mybir.AluOpType

B, T_FULL, D, H, L = 64, 256, 1024, 1024, 2
NCORES = 8
GROUPS = [[0, 1, 2, 3], [4, 5, 6, 7]]
CH = H // 4  # h-indices per core chunk (256)
KB = 8       # k-tiles (1024/128)
MB = 8       # m-tiles per core chunk (1024/128)

BF = ml_dtypes.bfloat16


def build(T: int):
    nc = bacc.Bacc("TRN2", target_bir_lowering=False, debug=False,
                   num_devices=NCORES)

    w_in = {}
    for l in range(L):
        w_in[("h", l)] = nc.dram_tensor(f"wh{l}", [128, KB * MB * 128], BF16,
                                        kind="ExternalInput")
        w_in[("x", l)] = nc.dram_tensor(f"wx{l}", [128, KB * MB * 128], BF16,
                                        kind="ExternalInput")
    b_in = [nc.dram_tensor(f"b{l}", [128, MB], F32, kind="ExternalInput")
            for l in range(L)]
    lenb_in = nc.dram_tensor("lenb", [128, 128], F32, kind="ExternalInput")
    xT_in = nc.dram_tensor("xT", [D, T * B], BF16, kind="ExternalInput")
    out1 = nc.dram_tensor("out1", [T, 128, 2, B], BF16, kind="ExternalOutput")

    with tile.TileContext(nc) as tc:
        with (
            tc.tile_pool(name="weights", bufs=1) as wpool,
            tc.tile_pool(name="state", bufs=1) as state,
            tc.tile_pool(name="work", bufs=3) as work,
            tc.tile_pool(name="psum", bufs=2, space="PSUM") as psum,
            tc.tile_pool(name="dram", bufs=3, space="DRAM") as dramp,
        ):
            # persistent tiles
            w_sb = {}
            for key, dram_t in w_in.items():
                w = wpool.tile([128, KB * MB * 128], BF16, tag=f"w{key}")
                nc.sync.dma_start(out=w[:, :], in_=dram_t[:, :])
                w_sb[key] = w
            b_sb = []
            for l in range(L):
                bt = state.tile([128, MB], F32, tag=f"b{l}")
                nc.sync.dma_start(out=bt[:, :], in_=b_in[l][:, :])
                b_sb.append(bt)
            lenb = state.tile([128, 128], F32, tag="lenb")
            nc.sync.dma_start(out=lenb[:, :], in_=lenb_in[:, :])

            h_st = [state.tile([128, 128], BF16, tag=f"h{l}", name=f"h{l}")
                    for l in range(L)]
            c_st = [state.tile([128, 128], F32, tag=f"c{l}", name=f"c{l}")
                    for l in range(L)]

            def wblk(kind, l, k, m):
                off = (k * MB + m) * 128
                return w_sb[(kind, l)][:, off:off + 128]

            def gemm(l, t, rhs_x, rhs_h):
                """PSUM tile = Wx_l^T x + Wh_l^T h (+ no bias; bias in ACT)."""
                ps = psum.tile([128, 512], F32, tag=f"ps{l}")
                for m in range(MB):
                    out = ps[:, 64 * m:64 * m + 64]
                    for k in range(KB):
                        nc.tensor.matmul(
                            out, wblk("x", l, k, m), rhs_x[:, 64 * k:64 * k + 64],
                            start=(k == 0),
                            stop=(rhs_h is None and k == KB - 1))
                    if rhs_h is not None:
                        for k in range(KB):
                            nc.tensor.matmul(
                                out, wblk("h", l, k, m),
                                rhs_h[:, 64 * k:64 * k + 64],
                                start=False, stop=(k == KB - 1))
                return ps

            def cell(l, t, ps):
                """LSTM cell elementwise; updates h_st[l], c_st[l] in place."""
                acts = work.tile([128, 512], BF16, tag=f"acts{l}")
                for m in range(MB):
                    func = AF.Sigmoid if m < 6 else AF.Tanh
                    nc.scalar.activation(
                        acts[:, 64 * m:64 * m + 64], ps[:, 64 * m:64 * m + 64],
                        func, bias=b_sb[l][:, m:m + 1])
                ig = acts[:, 0:128]
                fg = acts[:, 128:256]
                og = acts[:, 256:384]
                gg = acts[:, 384:512]
                h, c = h_st[l], c_st[l]
                tanh_c = work.tile([128, 128], BF16, tag=f"tanhc{l}")
                if t == 0:
                    # c = i*g ; h = o*tanh(c); lengths >= 1 so no mask at t=0
                    nc.vector.tensor_mul(c[:, :], ig, gg)
                    nc.scalar.activation(tanh_c[:, :], c[:, :], AF.Tanh)
                    nc.vector.tensor_mul(h[:, :], og, tanh_c[:, :])
                else:
                    v = work.tile([128, 128], mybir.dt.uint32, tag=f"v{l}")
                    nc.vector.tensor_single_scalar(v[:, :], lenb[:, :],
                                                   float(t), ALU.is_gt)
                    t1 = work.tile([128, 128], F32, tag=f"t1{l}")
                    nc.vector.tensor_mul(t1[:, :], fg, c[:, :])
                    t2 = work.tile([128, 128], BF16, tag=f"t2{l}")
                    nc.vector.tensor_mul(t2[:, :], ig, gg)
                    cn = work.tile([128, 128], F32, tag=f"cn{l}")
                    nc.vector.tensor_add(cn[:, :], t1[:, :], t2[:, :])
                    nc.vector.copy_predicated(c[:, :], v[:, :], cn[:, :])
                    nc.scalar.activation(tanh_c[:, :], c[:, :], AF.Tanh)
                    hn = work.tile([128, 128], BF16, tag=f"hn{l}")
                    nc.vector.tensor_mul(hn[:, :], og, tanh_c[:, :])
                    nc.vector.copy_predicated(h[:, :], v[:, :], hn[:, :])

            def allgather(l, t):
                """AG the h chunk; returns SBUF rhs tile [128, 8*64]."""
                agin = dramp.tile([128, 2, B], BF16, tag=f"agin{l}")
                nc.sync.dma_start(out=agin[:, :, :], in_=h_st[l][:, :])
                agout = dramp.tile([4, 128, 2, B], BF16, tag=f"agout{l}")
                nc.gpsimd.collective_compute(
                    "AllGather", ALU.bypass, replica_groups=GROUPS,
                    ins=[agin[:, :, :]], outs=[agout[:, :, :, :]])
                rhs = work.tile([128, 512], BF16, tag=f"rh{l}")
                nc.sync.dma_start(
                    out=rhs[:, :],
                    in_=agout.rearrange("r p k b -> p r k b"))
                return rhs

            xT_r = xT_in.rearrange("(k p) n -> p k n", p=128)

            # Wavefront: at global step g, layer 0 runs step g and layer 1
            # runs step g-1. rh0 holds the AG'd full h0_{g-1} — it is both
            # layer 0's h-part rhs at step g and layer 1's x-part rhs at
            # step g-1.
            rh0 = None
            rh1 = None
            for g in range(T + 1):
                rh0_next = None
                if g < T:
                    rx = work.tile([128, 512], BF16, tag="rx0")
                    nc.sync.dma_start(out=rx[:, :],
                                      in_=xT_r[:, :, B * g:B * (g + 1)])
                    ps0 = gemm(0, g, rx, rh0)
                    cell(0, g, ps0)
                    rh0_next = allgather(0, g)
                if g >= 1:
                    t = g - 1
                    ps1 = gemm(1, t, rh0, rh1)
                    cell(1, t, ps1)
                    nc.sync.dma_start(out=out1[t][:, :, :], in_=h_st[1][:, :])
                    if t < T - 1:
                        rh1 = allgather(1, t)
                if g < T:
                    rh0 = rh0_next
    nc.compile()
    return nc


# gate column bases in the reference's 4H axis: i, f, g, o
_GATE_BASE = {"i": 0, "f": H, "g": 2 * H, "o": 3 * H}
# m-tile order within a core's 1024-column chunk: [i0 i1 f0 f1 o0 o1 g0 g1]
_MTILE_GATES = ["i", "i", "f", "f", "o", "o", "g", "g"]


def _chunk_cols(j):
    """Column indices (into 4H) for core-chunk j, in m-tile order."""
    cols = []
    for m, gate in enumerate(_MTILE_GATES):
        blk = m % 2
        start = _GATE_BASE[gate] + CH * j + 128 * blk
        cols.extend(range(start, start + 128))
    return np.asarray(cols)


def _prep_w(W, cols):
    """W [K, 4H] fp32 -> lhsT tile layout [128, KB*MB*128] bf16.

    out[p, (k*MB+m)*128 + q] = W[128k + p, cols[128m + q]]
    """
    K = W.shape[0]
    Wc = W[:, cols]                                  # [K, 1024]
    Wc = Wc.reshape(K // 128, 128, MB, 128)           # [k, p, m, q]
    Wc = Wc.transpose(1, 0, 2, 3).reshape(128, -1)    # [p, k*m*q]
    return np.ascontiguousarray(Wc.astype(BF))


def _prep_inputs(x, lengths, fw_Wx, fw_Wh, fw_b, bw_Wx, bw_Wh, bw_b, T):
    x = np.asarray(x, np.float32)
    lengths = np.asarray(lengths)
    in_maps = []
    lenb = np.ascontiguousarray(
        np.tile(lengths.astype(np.float32)[None, :], (128, 2)))
    xt = x[:, :T, :]
    xT = {}
    for d, xd in enumerate([xt, xt[:, ::-1, :]]):
        # xT[dcol, t*B + b] = xd[b, t, dcol]
        xT[d] = np.ascontiguousarray(
            xd.transpose(2, 1, 0).reshape(D, T * B).astype(BF))
    Wx = [np.asarray(fw_Wx, np.float32), np.asarray(bw_Wx, np.float32)]
    Wh = [np.asarray(fw_Wh, np.float32), np.asarray(bw_Wh, np.float32)]
    bb = [np.asarray(fw_b, np.float32), np.asarray(bw_b, np.float32)]
    for core in range(NCORES):
        d, j = core // 4, core % 4
        cols = _chunk_cols(j)
        m = {"lenb": lenb, "xT": xT[d]}
        for l in range(L):
            m[f"wx{l}"] = _prep_w(Wx[d][l], cols)
            m[f"wh{l}"] = _prep_w(Wh[d][l], cols)
            m[f"b{l}"] = np.ascontiguousarray(
                bb[d][l][cols].reshape(MB, 128).T.astype(np.float32))
        in_maps.append(m)
    return in_maps


_NC_CACHE = {}


def _get_nc(T):
    if T not in _NC_CACHE:
        _NC_CACHE[T] = build(T)
    return _NC_CACHE[T]


def run(T=T_FULL, trace=False, **inputs):
    """Run the kernel at sequence length T; returns (outputs, final, results)."""
    nc = _get_nc(T)
    in_maps = _prep_inputs(T=T, **inputs)
    res = run_bass_kernel_spmd(nc, in_maps, core_ids=list(range(NCORES)),
                               trace=trace)
    # assemble: out1 [T, 128, 2, B] bf16 per core
    outputs = np.zeros((B, T, 2 * H), np.float32)
    for core in range(NCORES):
        d, j = core // 4, core % 4
        o = np.asarray(res.results[core]["out1"], np.float32)
        o = o.transpose(3, 0, 2, 1).reshape(B, T, CH)  # [b, t, 128K+p]
        if d == 1:
            o = o[:, ::-1, :]
        outputs[:, :, H * d + CH * j: H * d + CH * (j + 1)] = o
    final = np.concatenate([outputs[:, -1, :H], outputs[:, 0, H:]], axis=-1)
    return outputs, final, res


def kernel(**inputs):
    outputs, final, _ = run(T=T_FULL, **inputs)
    return outputs, final


# revision 15
# speedup vs baseline: 1.0218x; 1.0218x over previous
"""BiLSTM encoder (B=64, T=256, D=H=1024, L=2) on 8 Trainium2 NeuronCores.

Sharding: cores 0-3 run the forward direction, cores 4-7 the backward
direction (backward cores get time-reversed inputs so the program is
identical). Within each 4-core group, the 4H=4096 gate columns are sharded
into chunks of 1024 (= 256 h-indices x 4 gates); every core holds the full
batch of 64. Each recurrence step all-gathers the per-core 256-row h chunk
so every core has the full h for the next step's GEMM. Layer 1 runs
wavefronted one step behind layer 0 and consumes the layer-0 all-gather
output directly as its input GEMM operand.

Layout notes:
- All GEMMs are weight-stationary: out^T[gate_cols, batch] = W_block^T @ rhs
  with W blocks [128k, 128m] as lhsT and rhs [128, 64] slices streaming.
- Per-step PSUM tile [128, 512] holds 8 m-tiles in column order
  [i0 i1 f0 f1 o0 o1 g0 g1] (i/f/o/g gates, 2 h-blocks of 128 each).
- Cell state c is fp32, h and activations bf16.
"""

import sys

sys.path.insert(0, "/opt/trn_rl_repo")

import numpy as np
import ml_dtypes

import concourse.bass as bass
import concourse.bacc as bacc
import concourse.mybir as mybir
import concourse.tile as tile
from concourse.bass_utils import run_bass_kernel_spmd

F32 = mybir.dt.float32
BF16 = mybir.dt.bfloat16
AF = mybir.ActivationFunctionType
ALU = mybir.AluOpType

B, T_FULL, D, H, L = 64, 256, 1024, 1024, 2
NCORES = 8
GROUPS = [[0, 1, 2, 3], [4, 5, 6, 7]]
CH = H // 4  # h-indices per core chunk (256)
KB = 8       # k-tiles (1024/128)
MB = 8       # m-tiles per core chunk (1024/128)

BF = ml_dtypes.bfloat16


def build(T: int):
    nc = bacc.Bacc("TRN2", target_bir_lowering=False, debug=False,
                   num_devices=NCORES)

    w_in = {}
    for l in range(L):
        w_in[("h", l)] = nc.dram_tensor(f"wh{l}", [128, KB * MB * 128], BF16,
                                        kind="ExternalInput")
        w_in[("x", l)] = nc.dram_tensor(f"wx{l}", [128, KB * MB * 128], BF16,
                                        kind="ExternalInput")
    # bias broadcast over batch, matching the psum layout [p, 64m+b]
    b_in = [nc.dram_tensor(f"bb{l}", [128, 512], BF16, kind="ExternalInput")
            for l in range(L)]
    id_in = nc.dram_tensor("ident", [128, 128], BF16, kind="ExternalInput")
    lenb_in = nc.dram_tensor("lenb", [128, 128], F32, kind="ExternalInput")
    xT_in = nc.dram_tensor("xT", [D, T * B], BF16, kind="ExternalInput")
    out1 = nc.dram_tensor("out1", [T, 128, 2, B], BF16, kind="ExternalOutput")

    with tile.TileContext(nc) as tc:
        with (
            tc.tile_pool(name="weights", bufs=1) as wpool,
            tc.tile_pool(name="state", bufs=1) as state,
            tc.tile_pool(name="work", bufs=3) as work,
            tc.tile_pool(name="psum", bufs=2, space="PSUM") as psum,
            tc.tile_pool(name="dram", bufs=3, space="DRAM") as dramp,
        ):
            # persistent tiles
            w_sb = {}
            for key, dram_t in w_in.items():
                w = wpool.tile([128, KB * MB * 128], BF16, tag=f"w{key}")
                nc.sync.dma_start(out=w[:, :], in_=dram_t[:, :])
                w_sb[key] = w
            b_sb = []
            for l in range(L):
                bt = state.tile([128, 512], BF16, tag=f"b{l}", name=f"b{l}")
                nc.sync.dma_start(out=bt[:, :], in_=b_in[l][:, :])
                b_sb.append(bt)
            ident = state.tile([128, 128], BF16, tag="ident")
            nc.sync.dma_start(out=ident[:, :], in_=id_in[:, :])
            lenb = state.tile([128, 128], F32, tag="lenb")
            nc.sync.dma_start(out=lenb[:, :], in_=lenb_in[:, :])

            h_st = [state.tile([128, 128], BF16, tag=f"h{l}", name=f"h{l}")
                    for l in range(L)]
            c_st = [state.tile([128, 128], F32, tag=f"c{l}", name=f"c{l}")
                    for l in range(L)]

            def wblk(kind, l, k, m):
                off = (k * MB + m) * 128
                return w_sb[(kind, l)][:, off:off + 128]

            def gemm(l, t, rhs_x, rhs_h):
                """PSUM tile = bias + Wx_l^T x + Wh_l^T h."""
                ps = psum.tile([128, 512], F32, tag=f"ps{l}")
                # preload bias into the whole bank: psum = I^T @ bias_bcast
                nc.tensor.matmul(ps[:, :], ident[:, :], b_sb[l][:, :],
                                 start=True, stop=False, skip_group_check=True)
                for m in range(MB):
                    out = ps[:, 64 * m:64 * m + 64]
                    for k in range(KB):
                        nc.tensor.matmul(
                            out, wblk("x", l, k, m), rhs_x[:, 64 * k:64 * k + 64],
                            start=False,
                            stop=(rhs_h is None and k == KB - 1),
                            skip_group_check=True)
                    if rhs_h is not None:
                        for k in range(KB):
                            nc.tensor.matmul(
                                out, wblk("h", l, k, m),
                                rhs_h[:, 64 * k:64 * k + 64],
                                start=False, stop=(k == KB - 1),
                                skip_group_check=True)
                return ps

            def cell(l, t, ps):
                """LSTM cell elementwise; updates h_st[l], c_st[l] in place."""
                acts = work.tile([128, 512], BF16, tag=f"acts{l}")
                nc.scalar.activation(acts[:, 0:384], ps[:, 0:384], AF.Sigmoid)
                nc.scalar.activation(acts[:, 384:512], ps[:, 384:512], AF.Tanh)
                ig = acts[:, 0:128]
                fg = acts[:, 128:256]
                og = acts[:, 256:384]
                gg = acts[:, 384:512]
                h, c = h_st[l], c_st[l]
                tanh_c = work.tile([128, 128], BF16, tag=f"tanhc{l}")
                if t == 0:
                    # c = i*g ; h = o*tanh(c); lengths >= 1 so no mask at t=0
                    nc.vector.tensor_mul(c[:, :], ig, gg)
                    nc.scalar.activation(tanh_c[:, :], c[:, :], AF.Tanh)
                    nc.vector.tensor_mul(h[:, :], og, tanh_c[:, :])
                else:
                    # v first on DVE (no upstream deps), then the c chain
                    v = work.tile([128, 128], mybir.dt.uint32, tag=f"v{l}")
                    nc.vector.tensor_single_scalar(v[:, :], lenb[:, :],
                                                   float(t), ALU.is_gt)
                    t2 = work.tile([128, 128], BF16, tag=f"t2{l}")
                    nc.vector.tensor_mul(t2[:, :], ig, gg)
                    t1 = work.tile([128, 128], F32, tag=f"t1{l}")
                    nc.vector.tensor_mul(t1[:, :], fg, c[:, :])
                    cn = work.tile([128, 128], F32, tag=f"cn{l}")
                    nc.vector.tensor_add(cn[:, :], t1[:, :], t2[:, :])
                    nc.vector.copy_predicated(c[:, :], v[:, :], cn[:, :])
                    nc.scalar.activation(tanh_c[:, :], c[:, :], AF.Tanh)
                    hn = work.tile([128, 128], BF16, tag=f"hn{l}")
                    nc.vector.tensor_mul(hn[:, :], og, tanh_c[:, :])
                    nc.vector.copy_predicated(h[:, :], v[:, :], hn[:, :])

            def allgather(l, t):
                """AG the h chunk; returns SBUF rhs tile [128, 8*64]."""
                agin = dramp.tile([128, 2, B], BF16, tag=f"agin{l}")
                nc.sync.dma_start(out=agin[:, :, :], in_=h_st[l][:, :])
                agout = dramp.tile([4, 128, 2, B], BF16, tag=f"agout{l}")
                nc.gpsimd.collective_compute(
                    "AllGather", ALU.bypass, replica_groups=GROUPS,
                    ins=[agin[:, :, :]], outs=[agout[:, :, :, :]])
                rhs = work.tile([128, 512], BF16, tag=f"rh{l}")
                # per-rank contiguous DMAs; k-tile (2r+k) lands at col 64*(2r+k)
                for r in range(4):
                    nc.sync.dma_start(out=rhs[:, 128 * r:128 * (r + 1)],
                                      in_=agout[r])
                return rhs

            xT_r = xT_in.rearrange("(k p) n -> p k n", p=128)

            # Wavefront: at global step g, layer 0 runs step g and layer 1
            # runs step g-1. rh0 holds the AG'd full h0_{g-1} — it is both
            # layer 0's h-part rhs at step g and layer 1's x-part rhs at
            # step g-1.
            rh0 = None
            rh1 = None
            for g in range(T + 1):
                rh0_next = None
                if g < T:
                    rx = work.tile([128, 512], BF16, tag="rx0")
                    nc.sync.dma_start(out=rx[:, :],
                                      in_=xT_r[:, :, B * g:B * (g + 1)])
                    ps0 = gemm(0, g, rx, rh0)
                    cell(0, g, ps0)
                    rh0_next = allgather(0, g)
                if g >= 1:
                    t = g - 1
                    ps1 = gemm(1, t, rh0, rh1)
                    cell(1, t, ps1)
                    if t < T - 1:
                        rh1 = allgather(1, t)
                    nc.sync.dma_start(out=out1[t][:, :, :], in_=h_st[1][:, :])
                if g < T:
                    rh0 = rh0_next
    nc.compile()
    return nc


# gate column bases in the reference's 4H axis: i, f, g, o
_GATE_BASE = {"i": 0, "f": H, "g": 2 * H, "o": 3 * H}
# m-tile order within a core's 1024-column chunk: [i0 i1 f0 f1 o0 o1 g0 g1]
_MTILE_GATES = ["i", "i", "f", "f", "o", "o", "g", "g"]


def _chunk_cols(j):
    """Column indices (into 4H) for core-chunk j, in m-tile order."""
    cols = []
    for m, gate in enumerate(_MTILE_GATES):
        blk = m % 2
        start = _GATE_BASE[gate] + CH * j + 128 * blk
        cols.extend(range(start, start + 128))
    return np.asarray(cols)


def _prep_w(W, cols):
    """W [K, 4H] fp32 -> lhsT tile layout [128, KB*MB*128] bf16.

    out[p, (k*MB+m)*128 + q] = W[128k + p, cols[128m + q]]
    """
    K = W.shape[0]
    Wc = W[:, cols]                                  # [K, 1024]
    Wc = Wc.reshape(K // 128, 128, MB, 128)           # [k, p, m, q]
    Wc = Wc.transpose(1, 0, 2, 3).reshape(128, -1)    # [p, k*m*q]
    return np.ascontiguousarray(Wc.astype(BF))


def _prep_inputs(x, lengths, fw_Wx, fw_Wh, fw_b, bw_Wx, bw_Wh, bw_b, T):
    x = np.asarray(x, np.float32)
    lengths = np.asarray(lengths)
    in_maps = []
    lenb = np.ascontiguousarray(
        np.tile(lengths.astype(np.float32)[None, :], (128, 2)))
    xt = x[:, :T, :]
    xT = {}
    for d, xd in enumerate([xt, xt[:, ::-1, :]]):
        # xT[dcol, t*B + b] = xd[b, t, dcol]
        xT[d] = np.ascontiguousarray(
            xd.transpose(2, 1, 0).reshape(D, T * B).astype(BF))
    Wx = [np.asarray(fw_Wx, np.float32), np.asarray(bw_Wx, np.float32)]
    Wh = [np.asarray(fw_Wh, np.float32), np.asarray(bw_Wh, np.float32)]
    bb = [np.asarray(fw_b, np.float32), np.asarray(bw_b, np.float32)]
    ident = np.eye(128, dtype=BF)
    for core in range(NCORES):
        d, j = core // 4, core % 4
        cols = _chunk_cols(j)
        m = {"lenb": lenb, "xT": xT[d], "ident": ident}
        for l in range(L):
            m[f"wx{l}"] = _prep_w(Wx[d][l], cols)
            m[f"wh{l}"] = _prep_w(Wh[d][l], cols)
            # bias broadcast to psum layout [p, 64m + b]
            bm = bb[d][l][cols].reshape(MB, 128).T  # [p, m]
            m[f"bb{l}"] = np.ascontiguousarray(
                np.repeat(bm[:, :, None], B, axis=2).reshape(128, MB * B)
                .astype(BF))
        in_maps.append(m)
    return in_maps


_NC_CACHE = {}


def _get_nc(T):
    if T not in _NC_CACHE:
        _NC_CACHE[T] = build(T)
    return _NC_CACHE[T]


def run(T=T_FULL, trace=False, **inputs):
    """Run the kernel at sequence length T; returns (outputs, final, results)."""
    nc = _get_nc(T)
    in_maps = _prep_inputs(T=T, **inputs)
    res = run_bass_kernel_spmd(nc, in_maps, core_ids=list(range(NCORES)),
                               trace=trace)
    # assemble: out1 [T, 128, 2, B] bf16 per core
    outputs = np.zeros((B, T, 2 * H), np.float32)
    for core in range(NCORES):
        d, j = core // 4, core % 4
        o = np.asarray(res.results[core]["out1"], np.float32)
        o = o.transpose(3, 0, 2, 1).reshape(B, T, CH)  # [b, t, 128K+p]
        if d == 1:
            o = o[:, ::-1, :]
        outputs[:, :, H * d + CH * j: H * d + CH * (j + 1)] = o
    final = np.concatenate([outputs[:, -1, :H], outputs[:, 0, H:]], axis=-1)
    return outputs, final, res


def kernel(**inputs):
    outputs, final, _ = run(T=T_FULL, **inputs)
    return outputs, final


# revision 18
# speedup vs baseline: 1.0488x; 1.0264x over previous
"""BiLSTM encoder (B=64, T=256, D=H=1024, L=2) on 8 Trainium2 NeuronCores.

Sharding: cores 0-3 run the forward direction, cores 4-7 the backward
direction (backward cores get time-reversed inputs so the program is
identical). Within each 4-core group, the 4H=4096 gate columns are sharded
into chunks of 1024 (= 256 h-indices x 4 gates); every core holds the full
batch of 64. Each recurrence step all-gathers the per-core 256-row h chunk
so every core has the full h for the next step's GEMM. Layer 1 runs
wavefronted one step behind layer 0 and consumes the layer-0 all-gather
output directly as its input GEMM operand.

Layout notes:
- All GEMMs are weight-stationary: out^T[gate_cols, batch] = W_block^T @ rhs
  with W blocks [128k, 128m] as lhsT and rhs [128, 64] slices streaming.
- Per-step PSUM tile [128, 512] holds 8 m-tiles in column order
  [i0 i1 f0 f1 o0 o1 g0 g1] (i/f/o/g gates, 2 h-blocks of 128 each).
- Cell state c is fp32, h and activations bf16.
"""

import sys

sys.path.insert(0, "/opt/trn_rl_repo")

import numpy as np
import ml_dtypes

import concourse.bass as bass
import concourse.bacc as bacc
import concourse.mybir as mybir
import concourse.tile as tile
from concourse.bass_utils import run_bass_kernel_spmd

F32 = mybir.dt.float32
BF16 = mybir.dt.bfloat16
AF = mybir.ActivationFunctionType
ALU = mybir.AluOpType

B, T_FULL, D, H, L = 64, 256, 1024, 1024, 2
NCORES = 8
GROUPS = [[0, 1, 2, 3], [4, 5, 6, 7]]
CH = H // 4  # h-indices per core chunk (256)
KB = 8       # k-tiles (1024/128)
MB = 8       # m-tiles per core chunk (1024/128)

BF = ml_dtypes.bfloat16


def build(T: int):
    nc = bacc.Bacc("TRN2", target_bir_lowering=False, debug=False,
                   num_devices=NCORES)

    w_in = {}
    for l in range(L):
        w_in[("h", l)] = nc.dram_tensor(f"wh{l}", [128, KB * MB * 128], BF16,
                                        kind="ExternalInput")
        w_in[("x", l)] = nc.dram_tensor(f"wx{l}", [128, KB * MB * 128], BF16,
                                        kind="ExternalInput")
    # bias broadcast over batch, matching the psum layout [p, 64m+b]
    b_in = [nc.dram_tensor(f"bb{l}", [128, 512], BF16, kind="ExternalInput")
            for l in range(L)]
    id_in = nc.dram_tensor("ident", [128, 128], BF16, kind="ExternalInput")
    lenb_in = nc.dram_tensor("lenb", [128, 128], F32, kind="ExternalInput")
    xT_in = nc.dram_tensor("xT", [D, T * B], BF16, kind="ExternalInput")
    out1 = nc.dram_tensor("out1", [T, 128, 2, B], BF16, kind="ExternalOutput")

    with tile.TileContext(nc) as tc:
        with (
            tc.tile_pool(name="weights", bufs=1) as wpool,
            tc.tile_pool(name="state", bufs=1) as state,
            tc.tile_pool(name="work", bufs=4) as work,
            tc.tile_pool(name="psum", bufs=2, space="PSUM") as psum,
            tc.tile_pool(name="dram", bufs=3, space="DRAM") as dramp,
        ):
            # persistent tiles
            w_sb = {}
            for key, dram_t in w_in.items():
                w = wpool.tile([128, KB * MB * 128], BF16, tag=f"w{key}")
                nc.sync.dma_start(out=w[:, :], in_=dram_t[:, :])
                w_sb[key] = w
            b_sb = []
            for l in range(L):
                bt = state.tile([128, 512], BF16, tag=f"b{l}", name=f"b{l}")
                nc.sync.dma_start(out=bt[:, :], in_=b_in[l][:, :])
                b_sb.append(bt)
            ident = state.tile([128, 128], BF16, tag="ident")
            nc.sync.dma_start(out=ident[:, :], in_=id_in[:, :])
            lenb = state.tile([128, 128], F32, tag="lenb")
            nc.sync.dma_start(out=lenb[:, :], in_=lenb_in[:, :])

            h_st = [state.tile([128, 128], BF16, tag=f"h{l}", name=f"h{l}")
                    for l in range(L)]
            c_st = [state.tile([128, 128], F32, tag=f"c{l}", name=f"c{l}")
                    for l in range(L)]

            def wblk(kind, l, k, m):
                off = (k * MB + m) * 128
                return w_sb[(kind, l)][:, off:off + 128]

            def gemm(l, t, rhs_x, rhs_h):
                """PSUM tile = bias + Wx_l^T x + Wh_l^T h."""
                ps = psum.tile([128, 512], F32, tag=f"ps{l}")
                # preload bias into the whole bank: psum = I^T @ bias_bcast
                nc.tensor.matmul(ps[:, :], ident[:, :], b_sb[l][:, :],
                                 start=True, stop=False, skip_group_check=True)
                for m in range(MB):
                    out = ps[:, 64 * m:64 * m + 64]
                    for k in range(KB):
                        nc.tensor.matmul(
                            out, wblk("x", l, k, m), rhs_x[:, 64 * k:64 * k + 64],
                            start=False,
                            stop=(rhs_h is None and k == KB - 1),
                            skip_group_check=True)
                    if rhs_h is not None:
                        for k in range(KB):
                            nc.tensor.matmul(
                                out, wblk("h", l, k, m),
                                rhs_h[:, 64 * k:64 * k + 64],
                                start=False, stop=(k == KB - 1),
                                skip_group_check=True)
                return ps

            def cell(l, t, ps):
                """LSTM cell elementwise; updates h_st[l], c_st[l] in place."""
                acts = work.tile([128, 512], BF16, tag=f"acts{l}")
                nc.scalar.activation(acts[:, 0:384], ps[:, 0:384], AF.Sigmoid)
                nc.scalar.activation(acts[:, 384:512], ps[:, 384:512], AF.Tanh)
                ig = acts[:, 0:128]
                fg = acts[:, 128:256]
                og = acts[:, 256:384]
                gg = acts[:, 384:512]
                h, c = h_st[l], c_st[l]
                tanh_c = work.tile([128, 128], BF16, tag=f"tanhc{l}")
                if t == 0:
                    # c = i*g ; h = o*tanh(c); lengths >= 1 so no mask at t=0
                    nc.vector.tensor_mul(c[:, :], ig, gg)
                    nc.scalar.activation(tanh_c[:, :], c[:, :], AF.Tanh)
                    nc.vector.tensor_mul(h[:, :], og, tanh_c[:, :])
                else:
                    # v first on DVE (no upstream deps), then the c chain
                    v = work.tile([128, 128], mybir.dt.uint32, tag=f"v{l}")
                    nc.vector.tensor_single_scalar(v[:, :], lenb[:, :],
                                                   float(t), ALU.is_gt)
                    t2 = work.tile([128, 128], BF16, tag=f"t2{l}")
                    nc.vector.tensor_mul(t2[:, :], ig, gg)
                    t1 = work.tile([128, 128], F32, tag=f"t1{l}")
                    nc.vector.tensor_mul(t1[:, :], fg, c[:, :])
                    cn = work.tile([128, 128], F32, tag=f"cn{l}")
                    nc.vector.tensor_add(cn[:, :], t1[:, :], t2[:, :])
                    nc.vector.copy_predicated(c[:, :], v[:, :], cn[:, :])
                    nc.scalar.activation(tanh_c[:, :], c[:, :], AF.Tanh)
                    hn = work.tile([128, 128], BF16, tag=f"hn{l}")
                    nc.vector.tensor_mul(hn[:, :], og, tanh_c[:, :])
                    nc.vector.copy_predicated(h[:, :], v[:, :], hn[:, :])

            def allgather(l, t):
                """AG the h chunk; returns SBUF rhs tile [128, 8*64]."""
                agin = dramp.tile([128, 2, B], BF16, tag=f"agin{l}")
                nc.sync.dma_start(out=agin[:, :, :], in_=h_st[l][:, :])
                agout = dramp.tile([4, 128, 2, B], BF16, tag=f"agout{l}")
                nc.gpsimd.collective_compute(
                    "AllGather", ALU.bypass, replica_groups=GROUPS,
                    ins=[agin[:, :, :]], outs=[agout[:, :, :, :]])
                rhs = work.tile([128, 512], BF16, tag=f"rh{l}")
                # per-rank contiguous DMAs; k-tile (2r+k) lands at col 64*(2r+k)
                for r in range(4):
                    nc.sync.dma_start(out=rhs[:, 128 * r:128 * (r + 1)],
                                      in_=agout[r])
                return rhs

            xT_r = xT_in.rearrange("(k p) n -> p k n", p=128)

            # Wavefront with lag 2: at global step g, layer 0 runs step g and
            # layer 1 runs step g-2. All of layer 1's dependencies (AG0(g-2)
            # for its input GEMM, AG1(g-3) for its recurrent GEMM) are >= 1
            # step old at emission, so layer 1's whole block — including its
            # AllGather — runs concurrently with layer 0's AG-gated chain
            # instead of serializing behind it in the in-order engine queues.
            rh0a = None   # AG0 output of step g-1 (layer 0 h-part rhs)
            rh0b = None   # AG0 output of step g-2 (layer 1 x-part rhs)
            rh1 = None
            for g in range(T + 2):
                if g >= 2:
                    t = g - 2
                    ps1 = gemm(1, t, rh0b, rh1)
                    cell(1, t, ps1)
                    if t < T - 1:
                        rh1 = allgather(1, t)
                    nc.gpsimd.dma_start(out=out1[t][:, :, :], in_=h_st[1][:, :])
                if g < T:
                    rx = work.tile([128, 512], BF16, tag="rx0")
                    nc.gpsimd.dma_start(out=rx[:, :],
                                        in_=xT_r[:, :, B * g:B * (g + 1)])
                    ps0 = gemm(0, g, rx, rh0a)
                    cell(0, g, ps0)
                    rh0b = rh0a
                    rh0a = allgather(0, g)
                elif g == T:
                    rh0b = rh0a
    nc.compile()
    return nc


# gate column bases in the reference's 4H axis: i, f, g, o
_GATE_BASE = {"i": 0, "f": H, "g": 2 * H, "o": 3 * H}
# m-tile order within a core's 1024-column chunk: [i0 i1 f0 f1 o0 o1 g0 g1]
_MTILE_GATES = ["i", "i", "f", "f", "o", "o", "g", "g"]


def _chunk_cols(j):
    """Column indices (into 4H) for core-chunk j, in m-tile order."""
    cols = []
    for m, gate in enumerate(_MTILE_GATES):
        blk = m % 2
        start = _GATE_BASE[gate] + CH * j + 128 * blk
        cols.extend(range(start, start + 128))
    return np.asarray(cols)


def _prep_w(W, cols):
    """W [K, 4H] fp32 -> lhsT tile layout [128, KB*MB*128] bf16.

    out[p, (k*MB+m)*128 + q] = W[128k + p, cols[128m + q]]
    """
    K = W.shape[0]
    Wc = W[:, cols]                                  # [K, 1024]
    Wc = Wc.reshape(K // 128, 128, MB, 128)           # [k, p, m, q]
    Wc = Wc.transpose(1, 0, 2, 3).reshape(128, -1)    # [p, k*m*q]
    return np.ascontiguousarray(Wc.astype(BF))


def _prep_inputs(x, lengths, fw_Wx, fw_Wh, fw_b, bw_Wx, bw_Wh, bw_b, T):
    x = np.asarray(x, np.float32)
    lengths = np.asarray(lengths)
    in_maps = []
    lenb = np.ascontiguousarray(
        np.tile(lengths.astype(np.float32)[None, :], (128, 2)))
    xt = x[:, :T, :]
    xT = {}
    for d, xd in enumerate([xt, xt[:, ::-1, :]]):
        # xT[dcol, t*B + b] = xd[b, t, dcol]
        xT[d] = np.ascontiguousarray(
            xd.transpose(2, 1, 0).reshape(D, T * B).astype(BF))
    Wx = [np.asarray(fw_Wx, np.float32), np.asarray(bw_Wx, np.float32)]
    Wh = [np.asarray(fw_Wh, np.float32), np.asarray(bw_Wh, np.float32)]
    bb = [np.asarray(fw_b, np.float32), np.asarray(bw_b, np.float32)]
    ident = np.eye(128, dtype=BF)
    for core in range(NCORES):
        d, j = core // 4, core % 4
        cols = _chunk_cols(j)
        m = {"lenb": lenb, "xT": xT[d], "ident": ident}
        for l in range(L):
            m[f"wx{l}"] = _prep_w(Wx[d][l], cols)
            m[f"wh{l}"] = _prep_w(Wh[d][l], cols)
            # bias broadcast to psum layout [p, 64m + b]
            bm = bb[d][l][cols].reshape(MB, 128).T  # [p, m]
            m[f"bb{l}"] = np.ascontiguousarray(
                np.repeat(bm[:, :, None], B, axis=2).reshape(128, MB * B)
                .astype(BF))
        in_maps.append(m)
    return in_maps


_NC_CACHE = {}


def _get_nc(T):
    if T not in _NC_CACHE:
        _NC_CACHE[T] = build(T)
    return _NC_CACHE[T]


def run(T=T_FULL, trace=False, **inputs):
    """Run the kernel at sequence length T; returns (outputs, final, results)."""
    nc = _get_nc(T)
    in_maps = _prep_inputs(T=T, **inputs)
    res = run_bass_kernel_spmd(nc, in_maps, core_ids=list(range(NCORES)),
                               trace=trace)
    # assemble: out1 [T, 128, 2, B] bf16 per core
    outputs = np.zeros((B, T, 2 * H), np.float32)
    for core in range(NCORES):
        d, j = core // 4, core % 4
        o = np.asarray(res.results[core]["out1"], np.float32)
        o = o.transpose(3, 0, 2, 1).reshape(B, T, CH)  # [b, t, 128K+p]
        if d == 1:
            o = o[:, ::-1, :]
        outputs[:, :, H * d + CH * j: H * d + CH * (j + 1)] = o
    final = np.concatenate([outputs[:, -1, :H], outputs[:, 0, H:]], axis=-1)
    return outputs, final, res


def kernel(**inputs):
    outputs, final, _ = run(T=T_FULL, **inputs)
    return outputs, final


# revision 23
# speedup vs baseline: 1.2097x; 1.1534x over previous
"""BiLSTM encoder (B=64, T=256, D=H=1024, L=2) on 8 Trainium2 NeuronCores.

Sharding: cores 0-3 run the forward direction, cores 4-7 the backward
direction (backward cores get time-reversed inputs so the program is
identical). Within each 4-core group, the 4H=4096 gate columns are sharded
into chunks of 1024 (= 256 h-indices x 4 gates); every core holds the full
batch of 64. Each recurrence step all-gathers the per-core 256-row h chunk
so every core has the full h for the next step's GEMM. Layer 1 runs
wavefronted one step behind layer 0 and consumes the layer-0 all-gather
output directly as its input GEMM operand.

Layout notes:
- All GEMMs are weight-stationary: out^T[gate_cols, batch] = W_block^T @ rhs
  with W blocks [128k, 128m] as lhsT and rhs [128, 64] slices streaming.
- Per-step PSUM tile [128, 512] holds 8 m-tiles in column order
  [i0 i1 f0 f1 o0 o1 g0 g1] (i/f/o/g gates, 2 h-blocks of 128 each).
- Cell state c is fp32, h and activations bf16.
"""

import sys

sys.path.insert(0, "/opt/trn_rl_repo")

import numpy as np
import ml_dtypes

import concourse.bass as bass
import concourse.bacc as bacc
import concourse.mybir as mybir
import concourse.tile as tile
from concourse.bass_utils import run_bass_kernel_spmd

F32 = mybir.dt.float32
BF16 = mybir.dt.bfloat16
AF = mybir.ActivationFunctionType
ALU = mybir.AluOpType

B, T_FULL, D, H, L = 64, 256, 1024, 1024, 2
NCORES = 8
GROUPS = [[0, 1, 2, 3], [4, 5, 6, 7]]
CH = H // 4  # h-indices per core chunk (256)
KB = 8       # k-tiles (1024/128)
MB = 8       # m-tiles per core chunk (1024/128)

BF = ml_dtypes.bfloat16


def build(T: int):
    nc = bacc.Bacc("TRN2", target_bir_lowering=False, debug=False,
                   num_devices=NCORES)

    w_in = {}
    for l in range(L):
        w_in[("h", l)] = nc.dram_tensor(f"wh{l}", [128, KB * MB * 128], BF16,
                                        kind="ExternalInput")
        w_in[("x", l)] = nc.dram_tensor(f"wx{l}", [128, KB * MB * 128], BF16,
                                        kind="ExternalInput")
    # bias broadcast over batch, matching the psum layout [p, 64m+b]
    b_in = [nc.dram_tensor(f"bb{l}", [128, 512], BF16, kind="ExternalInput")
            for l in range(L)]
    id_in = nc.dram_tensor("ident", [128, 128], BF16, kind="ExternalInput")
    lenb_in = nc.dram_tensor("lenb", [128, 128], F32, kind="ExternalInput")
    xT_in = nc.dram_tensor("xT", [D, T * B], BF16, kind="ExternalInput")
    out1 = nc.dram_tensor("out1", [T, 128, 2, B], BF16, kind="ExternalOutput")

    with tile.TileContext(nc) as tc:
        with (
            tc.tile_pool(name="weights", bufs=1) as wpool,
            tc.tile_pool(name="state", bufs=1) as state,
            tc.tile_pool(name="work", bufs=4) as work,
            tc.tile_pool(name="psum", bufs=2, space="PSUM") as psum,
            tc.tile_pool(name="dram", bufs=3, space="DRAM") as dramp,
        ):
            # persistent tiles
            w_sb = {}
            for key, dram_t in w_in.items():
                w = wpool.tile([128, KB * MB * 128], BF16, tag=f"w{key}")
                nc.sync.dma_start(out=w[:, :], in_=dram_t[:, :])
                w_sb[key] = w
            b_sb = []
            for l in range(L):
                bt = state.tile([128, 512], BF16, tag=f"b{l}", name=f"b{l}")
                nc.sync.dma_start(out=bt[:, :], in_=b_in[l][:, :])
                b_sb.append(bt)
            ident = state.tile([128, 128], BF16, tag="ident")
            nc.sync.dma_start(out=ident[:, :], in_=id_in[:, :])
            lenb = state.tile([128, 128], F32, tag="lenb")
            nc.sync.dma_start(out=lenb[:, :], in_=lenb_in[:, :])

            h_st = [state.tile([128, 128], BF16, tag=f"h{l}", name=f"h{l}")
                    for l in range(L)]
            c_st = [state.tile([128, 128], F32, tag=f"c{l}", name=f"c{l}")
                    for l in range(L)]

            def wblk(kind, l, k, m):
                off = (k * MB + m) * 128
                return w_sb[(kind, l)][:, off:off + 128]

            def gemm(l, t, rhs_x, rhs_h):
                """PSUM tile = bias + Wx_l^T x + Wh_l^T h."""
                ps = psum.tile([128, 512], F32, tag=f"ps{l}")
                # preload bias into the whole bank: psum = I^T @ bias_bcast
                nc.tensor.matmul(ps[:, :], ident[:, :], b_sb[l][:, :],
                                 start=True, stop=False, skip_group_check=True)
                for m in range(MB):
                    out = ps[:, 64 * m:64 * m + 64]
                    for k in range(KB):
                        nc.tensor.matmul(
                            out, wblk("x", l, k, m), rhs_x[:, 64 * k:64 * k + 64],
                            start=False,
                            stop=(rhs_h is None and k == KB - 1),
                            skip_group_check=True)
                    if rhs_h is not None:
                        for k in range(KB):
                            nc.tensor.matmul(
                                out, wblk("h", l, k, m),
                                rhs_h[:, 64 * k:64 * k + 64],
                                start=False, stop=(k == KB - 1),
                                skip_group_check=True)
                return ps

            def cell(l, t, ps):
                """LSTM cell elementwise; updates h_st[l], c_st[l] in place."""
                acts = work.tile([128, 512], BF16, tag=f"acts{l}")
                nc.scalar.activation(acts[:, 0:384], ps[:, 0:384], AF.Sigmoid)
                nc.scalar.activation(acts[:, 384:512], ps[:, 384:512], AF.Tanh)
                ig = acts[:, 0:128]
                fg = acts[:, 128:256]
                og = acts[:, 256:384]
                gg = acts[:, 384:512]
                h, c = h_st[l], c_st[l]
                tanh_c = work.tile([128, 128], BF16, tag=f"tanhc{l}")
                if t == 0:
                    # c = i*g ; h = o*tanh(c); lengths >= 1 so no mask at t=0
                    nc.vector.tensor_mul(c[:, :], ig, gg)
                    nc.scalar.activation(tanh_c[:, :], c[:, :], AF.Tanh)
                    nc.vector.tensor_mul(h[:, :], og, tanh_c[:, :])
                else:
                    # v first on DVE (no upstream deps), then the c chain
                    v = work.tile([128, 128], mybir.dt.uint32, tag=f"v{l}")
                    nc.vector.tensor_single_scalar(v[:, :], lenb[:, :],
                                                   float(t), ALU.is_gt)
                    t2 = work.tile([128, 128], BF16, tag=f"t2{l}")
                    nc.vector.tensor_mul(t2[:, :], ig, gg)
                    t1 = work.tile([128, 128], F32, tag=f"t1{l}")
                    nc.vector.tensor_mul(t1[:, :], fg, c[:, :])
                    cn = work.tile([128, 128], F32, tag=f"cn{l}")
                    nc.vector.tensor_add(cn[:, :], t1[:, :], t2[:, :])
                    nc.vector.copy_predicated(c[:, :], v[:, :], cn[:, :])
                    nc.scalar.activation(tanh_c[:, :], c[:, :], AF.Tanh)
                    hn = work.tile([128, 128], BF16, tag=f"hn{l}")
                    nc.vector.tensor_mul(hn[:, :], og, tanh_c[:, :])
                    nc.vector.copy_predicated(h[:, :], v[:, :], hn[:, :])

            def allgather(l, t):
                """AG the h chunk; returns SBUF rhs tile [128, 8*64].

                Each layer gets its own DMA/trigger queue (L0 on the sync
                engine, L1 on gpsimd): the post-AG rhs DMA blocks its queue
                until the collective completes, so sharing one queue across
                layers would serialize the two layers' pipelines.
                """
                eng = nc.sync if l == 0 else nc.gpsimd
                agin = dramp.tile([128, 2, B], BF16, tag=f"agin{l}",
                                  name=f"agin{l}")
                eng.dma_start(out=agin[:, :, :], in_=h_st[l][:, :])
                agout = dramp.tile([4, 128, 2, B], BF16, tag=f"agout{l}",
                                   name=f"agout{l}")
                nc.gpsimd.collective_compute(
                    "AllGather", ALU.bypass, replica_groups=GROUPS,
                    ins=[agin[:, :, :]], outs=[agout[:, :, :, :]])
                rhs = work.tile([128, 512], BF16, tag=f"rh{l}", name=f"rh{l}")
                eng.dma_start(out=rhs[:, :],
                              in_=agout.rearrange("r p k b -> p r k b"))
                return rhs

            xT_r = xT_in.rearrange("(k p) n -> p k n", p=128)

            # Wavefront with lag 2: at global step g, layer 0 runs step g and
            # layer 1 runs step g-2. All of layer 1's dependencies (AG0(g-2)
            # for its input GEMM, AG1(g-3) for its recurrent GEMM) are >= 1
            # step old at emission, so layer 1's whole block — including its
            # AllGather — runs concurrently with layer 0's AG-gated chain
            # instead of serializing behind it in the in-order engine queues.
            rh0a = None   # AG0 output of step g-1 (layer 0 h-part rhs)
            rh0b = None   # AG0 output of step g-2 (layer 1 x-part rhs)
            rh1 = None
            for g in range(T + 2):
                if g >= 2:
                    t = g - 2
                    ps1 = gemm(1, t, rh0b, rh1)
                    cell(1, t, ps1)
                    if t < T - 1:
                        rh1 = allgather(1, t)
                    nc.scalar.dma_start(out=out1[t][:, :, :], in_=h_st[1][:, :])
                if g < T:
                    rx = work.tile([128, 512], BF16, tag="rx0")
                    nc.scalar.dma_start(out=rx[:, :],
                                        in_=xT_r[:, :, B * g:B * (g + 1)])
                    ps0 = gemm(0, g, rx, rh0a)
                    cell(0, g, ps0)
                    rh0b = rh0a
                    rh0a = allgather(0, g)
                elif g == T:
                    rh0b = rh0a
    nc.compile()
    return nc


# gate column bases in the reference's 4H axis: i, f, g, o
_GATE_BASE = {"i": 0, "f": H, "g": 2 * H, "o": 3 * H}
# m-tile order within a core's 1024-column chunk: [i0 i1 f0 f1 o0 o1 g0 g1]
_MTILE_GATES = ["i", "i", "f", "f", "o", "o", "g", "g"]


def _chunk_cols(j):
    """Column indices (into 4H) for core-chunk j, in m-tile order."""
    cols = []
    for m, gate in enumerate(_MTILE_GATES):
        blk = m % 2
        start = _GATE_BASE[gate] + CH * j + 128 * blk
        cols.extend(range(start, start + 128))
    return np.asarray(cols)


def _prep_w(W, cols):
    """W [K, 4H] fp32 -> lhsT tile layout [128, KB*MB*128] bf16.

    out[p, (k*MB+m)*128 + q] = W[128k + p, cols[128m + q]]
    """
    K = W.shape[0]
    Wc = W[:, cols]                                  # [K, 1024]
    Wc = Wc.reshape(K // 128, 128, MB, 128)           # [k, p, m, q]
    Wc = Wc.transpose(1, 0, 2, 3).reshape(128, -1)    # [p, k*m*q]
    return np.ascontiguousarray(Wc.astype(BF))


def _prep_inputs(x, lengths, fw_Wx, fw_Wh, fw_b, bw_Wx, bw_Wh, bw_b, T):
    x = np.asarray(x, np.float32)
    lengths = np.asarray(lengths)
    in_maps = []
    lenb = np.ascontiguousarray(
        np.tile(lengths.astype(np.float32)[None, :], (128, 2)))
    xt = x[:, :T, :]
    xT = {}
    for d, xd in enumerate([xt, xt[:, ::-1, :]]):
        # xT[dcol, t*B + b] = xd[b, t, dcol]
        xT[d] = np.ascontiguousarray(
            xd.transpose(2, 1, 0).reshape(D, T * B).astype(BF))
    Wx = [np.asarray(fw_Wx, np.float32), np.asarray(bw_Wx, np.float32)]
    Wh = [np.asarray(fw_Wh, np.float32), np.asarray(bw_Wh, np.float32)]
    bb = [np.asarray(fw_b, np.float32), np.asarray(bw_b, np.float32)]
    ident = np.eye(128, dtype=BF)
    for core in range(NCORES):
        d, j = core // 4, core % 4
        cols = _chunk_cols(j)
        m = {"lenb": lenb, "xT": xT[d], "ident": ident}
        for l in range(L):
            m[f"wx{l}"] = _prep_w(Wx[d][l], cols)
            m[f"wh{l}"] = _prep_w(Wh[d][l], cols)
            # bias broadcast to psum layout [p, 64m + b]
            bm = bb[d][l][cols].reshape(MB, 128).T  # [p, m]
            m[f"bb{l}"] = np.ascontiguousarray(
                np.repeat(bm[:, :, None], B, axis=2).reshape(128, MB * B)
                .astype(BF))
        in_maps.append(m)
    return in_maps


_NC_CACHE = {}


def _get_nc(T):
    if T not in _NC_CACHE:
        _NC_CACHE[T] = build(T)
    return _NC_CACHE[T]


def run(T=T_FULL, trace=False, **inputs):
    """Run the kernel at sequence length T; returns (outputs, final, results)."""
    nc = _get_nc(T)
    in_maps = _prep_inputs(T=T, **inputs)
    res = run_bass_kernel_spmd(nc, in_maps, core_ids=list(range(NCORES)),
                               trace=trace)
    # assemble: out1 [T, 128, 2, B] bf16 per core
    outputs = np.zeros((B, T, 2 * H), np.float32)
    for core in range(NCORES):
        d, j = core // 4, core % 4
        o = np.asarray(res.results[core]["out1"], np.float32)
        o = o.transpose(3, 0, 2, 1).reshape(B, T, CH)  # [b, t, 128K+p]
        if d == 1:
            o = o[:, ::-1, :]
        outputs[:, :, H * d + CH * j: H * d + CH * (j + 1)] = o
    final = np.concatenate([outputs[:, -1, :H], outputs[:, 0, H:]], axis=-1)
    return outputs, final, res


def kernel(**inputs):
    outputs, final, _ = run(T=T_FULL, **inputs)
    return outputs, final
